# revision 15
# baseline (speedup 1.0000x reference)
"""DiffusionGPT Trainium2 kernel (bf16, pipelined).

Data-parallel over batch: 8 batch elements -> 8 NeuronCores.  Activations
feature-major ([feature partitions, token free]) in bf16; weights host-
converted to bf16.  LayerNorm gains/biases are folded into the following
matmul weights on the host, so on-device LN is just (x - mean) * rstd.

PE-stream pipelining (the PE engine executes in program order, so emission
order is schedule):
  - attention is head-pipelined: att@V for head h-1 is emitted after
    scores+exp of head h, so the PE never head-of-line blocks on the exp.
  - proj(c0) is emitted in the middle of attention chunk 1 to fill the
    PE idle while the Act engine chews exp.
  - LN1 of layer l+1 runs between the two MLP chunks of layer l, hiding
    the LN stats chain under MLP matmuls.
  - v->token-major transposes use a 2-buffer PSUM ring so the PE does not
    serialize behind the DVE drain of each transpose.
PSUM drains carry the bias adds and run on DVE (att/gelu keep Act busy);
residual adds are fused (psum + bias) + x scalar_tensor_tensor ops.

Shapes: B=8, T=1022, S=1024, E=512, H=8 heads, D=64, F=2048, L=4 layers.
"""

import sys

sys.path.insert(0, "/opt/trn_rl_repo")

from contextlib import ExitStack

import numpy as np
import ml_dtypes

import concourse.bass as bass
import concourse.bacc as bacc
import concourse.tile as tile
from concourse import mybir
from concourse.bass_utils import run_bass_kernel_spmd
from concourse.masks import make_identity
from concourse import library_config

F32 = mybir.dt.float32
BF16 = mybir.dt.bfloat16
AF = mybir.ActivationFunctionType
ALU = mybir.AluOpType
BF = ml_dtypes.bfloat16

B = 8
T = 1022
S = 1024
E = 512
H = 8
D = 64
F = 2048
L = 4
NT = E // 128      # 4 feature tiles
NTT = S // 128     # 8 token tiles
LN_EPS = 1e-5
SCALE = 1.0 / 8.0  # 1/sqrt(D)

CHUNKS = ((0, 512), (512, 1024))


def build_nc(num_layers=L, do_head=True):
    nc = bacc.Bacc("TRN2", target_bir_lowering=False, debug=False)

    # ---- DRAM I/O ----
    d_sa = nc.dram_tensor("state_actions", [T, 72], F32, kind="ExternalInput")
    d_goals = nc.dram_tensor("goals", [1, 3], F32, kind="ExternalInput")
    d_sigma = nc.dram_tensor("sigma", [1], F32, kind="ExternalInput")
    d_sigma_w = nc.dram_tensor("sigma_w", [1, E], F32, kind="ExternalInput")
    d_sigma_b = nc.dram_tensor("sigma_b", [E], F32, kind="ExternalInput")
    d_tok_w = nc.dram_tensor("tok_w", [72, E], F32, kind="ExternalInput")
    d_tok_b = nc.dram_tensor("tok_b", [E], F32, kind="ExternalInput")
    d_goal_w = nc.dram_tensor("goal_w", [3, E], F32, kind="ExternalInput")
    d_goal_b = nc.dram_tensor("goal_b", [E], F32, kind="ExternalInput")
    d_pos = nc.dram_tensor("pos_emb", [1, S, E], F32, kind="ExternalInput")
    # LN-folded effective biases
    d_q_b = nc.dram_tensor("q_be", [L, E], F32, kind="ExternalInput")
    d_k_b = nc.dram_tensor("k_be", [L, E], F32, kind="ExternalInput")
    d_v_b = nc.dram_tensor("v_be", [L, E], F32, kind="ExternalInput")
    d_proj_b = nc.dram_tensor("proj_be", [L, E], F32, kind="ExternalInput")
    d_b1 = nc.dram_tensor("mlp_b1e", [L, F], F32, kind="ExternalInput")
    d_b2 = nc.dram_tensor("mlp_b2e", [L, E], F32, kind="ExternalInput")
    d_pred_b = nc.dram_tensor("pred_be", [72], F32, kind="ExternalInput")
    # host-converted, LN-gain-folded bf16 weights
    d_q_w = nc.dram_tensor("q_w16", [L, E, E], BF16, kind="ExternalInput")
    d_k_w = nc.dram_tensor("k_w16", [L, E, E], BF16, kind="ExternalInput")
    d_v_w = nc.dram_tensor("v_w16", [L, E, E], BF16, kind="ExternalInput")
    d_proj_w = nc.dram_tensor("proj_w16", [L, E, E], BF16, kind="ExternalInput")
    d_w1 = nc.dram_tensor("mlp_w116", [L, E, F], BF16, kind="ExternalInput")
    d_w2 = nc.dram_tensor("mlp_w216", [L, F, E], BF16, kind="ExternalInput")
    d_pred_w = nc.dram_tensor("pred_w16", [E, 72], BF16, kind="ExternalInput")
    d_out = nc.dram_tensor("out", [T, 72], F32, kind="ExternalOutput")

    with tile.TileContext(nc) as tc, ExitStack() as ctx:
        nc.gpsimd.load_library(library_config.attnmlp)

        const = ctx.enter_context(tc.tile_pool(name="const", bufs=1))
        big = ctx.enter_context(tc.tile_pool(name="big", bufs=1))
        wqkv = ctx.enter_context(tc.tile_pool(name="wqkv", bufs=28))
        w1p = ctx.enter_context(tc.tile_pool(name="w1p", bufs=6))
        w2p = ctx.enter_context(tc.tile_pool(name="w2p", bufs=18))
        bmat = ctx.enter_context(tc.tile_pool(name="bmat", bufs=2))
        bcols = ctx.enter_context(tc.tile_pool(name="bcols", bufs=8))
        ptp = ctx.enter_context(tc.tile_pool(name="ptp", bufs=8))
        usp = ctx.enter_context(tc.tile_pool(name="usp", bufs=16))
        rowp = ctx.enter_context(tc.tile_pool(name="rowp", bufs=5))
        bbp = ctx.enter_context(tc.tile_pool(name="bbp", bufs=2))
        recp = ctx.enter_context(tc.tile_pool(name="recp", bufs=2))
        scr = ctx.enter_context(tc.tile_pool(name="scr", bufs=4))

        # PSUM (16KB/partition): sc 2x4KB + b 2x2KB + o (2KB+.25) + t 2x.25
        ps_sc = ctx.enter_context(tc.tile_pool(name="ps_sc", bufs=2, space="PSUM"))
        ps_b = ctx.enter_context(tc.tile_pool(name="ps_b", bufs=2, space="PSUM"))
        ps_o = ctx.enter_context(tc.tile_pool(name="ps_o", bufs=1, space="PSUM"))
        ps_t = ps_o
        ps_st = ps_b

        # ---- constants ----
        ident = const.tile([128, 128], F32)
        make_identity(nc, ident[:])
        ident16 = const.tile([128, 128], BF16)
        nc.vector.tensor_copy(ident16[:], ident[:])
        ones64_16 = const.tile([128, 64], BF16)
        nc.gpsimd.memset(ones64_16[:], 1.0)
        ones_col16 = ones64_16[:, 0:1]

        # persistent activations (feature-major bf16)
        x_t = [big.tile([128, S], BF16, name=f"x{i}") for i in range(NT)]
        h_t = [big.tile([128, S], BF16, name=f"h{i}") for i in range(NT)]
        q_t = [big.tile([128, S], BF16, name=f"qa{i}") for i in range(NT)]
        k_t = [big.tile([128, S], BF16, name=f"ka{i}") for i in range(NT)]
        y_t = h_t
        sq_t = [big.tile([128, 512], BF16, name=f"sq{i}") for i in range(NT)]
        # v token-major with ones column per head: [128 tok, 8 heads x 65]
        vtok = [big.tile([128, H * 65], BF16, name=f"vtok{i}") for i in range(NTT)]
        for kt in range(NTT):
            vt3 = vtok[kt].rearrange("p (h c) -> p h c", c=65)
            nc.vector.tensor_copy(
                vt3[:, :, 64:65],
                ones64_16.rearrange("p (b c) -> p b c", c=1)[:, 0:H, :])

        # =================================================================
        # Embedding (f32 path, writes bf16 x)
        # =================================================================
        ones16_row = const.tile([1, 1024], BF16)
        nc.gpsimd.memset(ones16_row[:], 1.0)
        saT = const.tile([73, T], BF16)
        # engines can't start at partition 72; DMA can write any partition
        nc.sync.dma_start(saT[72:73, :], ones16_row[:, 0:T])
        for tt in range(NTT):
            ntt = min(128, T - tt * 128)
            sa_tok = scr.tile([128, 72], F32, tag="sa_tok")
            nc.sync.dma_start(sa_tok[0:ntt, :], d_sa[tt * 128: tt * 128 + ntt, :])
            sa16 = scr.tile([128, 72], BF16, tag="sa16", bufs=2)
            nc.vector.tensor_copy(sa16[0:ntt, :], sa_tok[0:ntt, :])
            tp = ps_t.tile([128, 128], BF16, tag="tpb", bufs=1)
            nc.tensor.matmul(tp[0:72, 0:ntt], sa16[0:ntt, 0:72],
                             ident16[0:ntt, 0:ntt], is_transpose=True)
            nc.vector.tensor_copy(saT[0:72, tt * 128: tt * 128 + ntt], tp[0:72, 0:ntt])

        tokw_f = const.tile([73, E], F32)
        nc.sync.dma_start(tokw_f[0:72, :], d_tok_w[:, :])
        nc.sync.dma_start(tokw_f[72:73, :], d_tok_b.ap().rearrange("(a e) -> a e", a=1))
        tokw_aug = const.tile([73, E], BF16)
        nc.vector.tensor_copy(tokw_aug[:], tokw_f[:])

        # sigma & goal columns: K=7 matmul per feature tile
        G_sf = const.tile([7, E], F32)
        nc.sync.dma_start(G_sf[0:1, :], d_sigma_w[:, :])
        nc.sync.dma_start(G_sf[1:2, :], d_sigma_b.ap().rearrange("(a e) -> a e", a=1))
        nc.sync.dma_start(G_sf[2:5, :], d_goal_w[:, :])
        nc.sync.dma_start(G_sf[5:6, :], d_goal_b.ap().rearrange("(a e) -> a e", a=1))
        nc.sync.dma_start(G_sf[6:7, :], d_pos.ap()[0, 0:1, :])
        G_sb = const.tile([7, E], BF16)
        nc.vector.tensor_copy(G_sb[:], G_sf[:])

        sig_sb = const.tile([1, 1], F32)
        nc.sync.dma_start(sig_sb[:], d_sigma.ap().rearrange("(a e) -> a e", a=1))
        lns = const.tile([1, 1], F32)
        nc.scalar.activation(lns[:], sig_sb[:], AF.Ln)
        sg_rowf = const.tile([1, 14], F32)
        nc.gpsimd.memset(sg_rowf[:], 0.0)
        nc.scalar.activation(sg_rowf[0:1, 0:1], lns[:], AF.Copy, scale=0.25)
        nc.gpsimd.memset(sg_rowf[0:1, 1:2], 1.0)
        g_row = const.tile([1, 3], F32)
        nc.sync.dma_start(g_row[:], d_goals[:, :])
        nc.vector.tensor_copy(sg_rowf[0:1, 9:12], g_row[:])
        nc.gpsimd.memset(sg_rowf[0:1, 12:14], 1.0)
        sg_row = const.tile([1, 14], BF16)
        nc.vector.tensor_copy(sg_row[:], sg_rowf[:])
        sg_rhs = const.tile([7, 2], BF16)
        for col in range(2):
            gtp = ps_t.tile([128, 128], BF16, tag="tpb", bufs=1)
            nc.tensor.matmul(gtp[0:7, 0:1], sg_row[0:1, col * 7:(col + 1) * 7],
                             ident16[0:1, 0:1], is_transpose=True)
            nc.vector.tensor_copy(sg_rhs[:, col: col + 1], gtp[0:7, 0:1])

        for fc in range(NT):
            sg_ps = ps_o.tile([128, 512], F32, tag="o")
            nc.tensor.matmul(sg_ps[0:128, 0:2], G_sb[:, fc * 128:(fc + 1) * 128],
                             sg_rhs[:], start=True, stop=True)
            nc.scalar.activation(x_t[fc][:, 0:2], sg_ps[0:128, 0:2], AF.Copy)

        # sa tokens -> x columns 2..1023
        for tt in range(NTT):
            ntt = min(128, T - tt * 128)
            xe_ps = ps_b.tile([128, 512], F32, tag="b")
            nc.tensor.matmul(xe_ps[0:ntt, :], saT[:, tt * 128: tt * 128 + ntt],
                             tokw_aug[:], start=True, stop=True)
            pos_sb = ptp.tile([128, E], F32, tag="pT", bufs=2)
            nc.sync.dma_start(pos_sb[0:ntt, :],
                              d_pos.ap()[0, tt * 128 + 1: tt * 128 + 1 + ntt, :])
            xe_tok = ptp.tile([128, E], BF16, tag="pT16", bufs=2)
            nc.vector.tensor_add(xe_tok[0:ntt, :], xe_ps[0:ntt, :], pos_sb[0:ntt, :])
            for fc in range(NT):
                tp = ps_t.tile([128, 128], BF16, tag="tpb", bufs=1)
                nc.tensor.matmul(tp[:, 0:ntt],
                                 xe_tok[0:ntt, fc * 128:(fc + 1) * 128],
                                 ident16[0:ntt, 0:ntt], is_transpose=True)
                nc.vector.tensor_copy(
                    x_t[fc][:, 2 + tt * 128: 2 + tt * 128 + ntt], tp[:, 0:ntt])

        # =================================================================
        # helpers
        # =================================================================
        def ln_chunk(c, src_t, dst_t):
            """Folded LayerNorm of chunk c: dst = (src - mean) * rstd."""
            c0, c1 = CHUNKS[c]
            for ti in range(NT):
                nc.vector.tensor_mul(sq_t[ti][:], src_t[ti][:, c0:c1],
                                     src_t[ti][:, c0:c1])
            s1t = ps_st.tile([128, 512], F32, tag="b")
            s1 = s1t[0:1, :]
            for ti in range(NT):
                nc.tensor.matmul(s1, ones_col16, src_t[ti][:, c0:c1],
                                 start=(ti == 0), stop=(ti == NT - 1))
            mean_row = rowp.tile([1, 512], F32, tag="rows")
            nc.vector.tensor_scalar_mul(mean_row[:], s1, 1.0 / E)
            s2t = ps_st.tile([128, 512], F32, tag="b")
            s2 = s2t[0:1, :]
            for ti in range(NT):
                nc.tensor.matmul(s2, ones_col16, sq_t[ti][:],
                                 start=(ti == 0), stop=(ti == NT - 1))
            m2_row = rowp.tile([1, 512], F32, tag="rows")
            nc.vector.tensor_scalar(m2_row[:], s2, 1.0 / E, LN_EPS,
                                    ALU.mult, ALU.add)
            msq = rowp.tile([1, 512], F32, tag="rows")
            nc.vector.tensor_mul(msq[:], mean_row[:], mean_row[:])
            nc.vector.tensor_sub(m2_row[:], m2_row[:], msq[:])
            w_row = rowp.tile([1, 512], F32, tag="rows")
            nc.vector.reciprocal(w_row[:], m2_row[:])
            rstd_row = rowp.tile([1, 512], F32, tag="rows")
            nc.scalar.activation(rstd_row[:], w_row[:], AF.Sqrt)
            mb = bbp.tile([128, 512], F32, tag="bb")
            nc.gpsimd.partition_broadcast(mb[:], mean_row[:])
            rb = bbp.tile([128, 512], F32, tag="bb")
            nc.gpsimd.partition_broadcast(rb[:], rstd_row[:])
            for ti in range(NT):
                t0 = sq_t[ti]  # reuse square scratch as LN scratch
                nc.vector.tensor_sub(t0[:], src_t[ti][:, c0:c1], mb[:])
                nc.vector.tensor_mul(dst_t[ti][:, c0:c1], t0[:], rb[:])

        def matmul_out(c, w_sb, in_t):
            """yield (ot, psum tile) for out = w^T @ in over chunk c.
            Alternates between the o/b PSUM rings for depth-3 pipelining."""
            c0, c1 = CHUNKS[c]
            for ot in range(NT):
                if ot % 2 == 0:
                    ps = ps_o.tile([128, 512], F32, tag="o")
                else:
                    ps = ps_b.tile([128, 512], F32, tag="b")
                for kc in range(NT):
                    nc.tensor.matmul(
                        ps[:], w_sb[kc][:, ot * 128:(ot + 1) * 128],
                        in_t[kc][:, c0:c1],
                        start=(kc == 0), stop=(kc == NT - 1))
                yield ot, ps

        # =================================================================
        # Transformer layers
        # =================================================================
        def load_qkvp(l):
            res = []
            for dw in (d_q_w, d_k_w, d_v_w, d_proj_w):
                tiles = []
                for kc in range(NT):
                    wt = wqkv.tile([128, E], BF16, tag="w")
                    nc.sync.dma_start(wt[:], dw.ap()[l, kc * 128:(kc + 1) * 128, :])
                    tiles.append(wt)
                res.append(tiles)
            return res

        def load_w1(l):
            tiles = []
            for kc in range(NT):
                wt = w1p.tile([128, F], BF16, tag="w1")
                nc.sync.dma_start(wt[:], d_w1.ap()[l, kc * 128:(kc + 1) * 128, :])
                tiles.append(wt)
            return tiles

        def load_w2(l):
            tiles = []
            for h16 in range(F // 128):
                wt = w2p.tile([128, E], BF16, tag="w2")
                nc.sync.dma_start(wt[:], d_w2.ap()[l, h16 * 128:(h16 + 1) * 128, :])
                tiles.append(wt)
            return tiles

        def load_bias_cols(l):
            # rows: 0 q_b, 1 k_b, 2 v_b, 3 proj_b, 4 mlp_b2, 5..8 mlp_b1
            Bm = bmat.tile([9, E], F32, tag="B")
            nc.sync.dma_start(Bm[0:1, :], d_q_b.ap()[l: l + 1, :])
            nc.sync.dma_start(Bm[1:2, :], d_k_b.ap()[l: l + 1, :])
            nc.sync.dma_start(Bm[2:3, :], d_v_b.ap()[l: l + 1, :])
            nc.sync.dma_start(Bm[3:4, :], d_proj_b.ap()[l: l + 1, :])
            nc.sync.dma_start(Bm[4:5, :], d_b2.ap()[l: l + 1, :])
            nc.sync.dma_start(Bm[5:9, :],
                              d_b1.ap()[l: l + 1, :].rearrange("a (b e) -> (a b) e", e=E))
            Bm16 = bmat.tile([9, E], BF16, tag="B16")
            nc.vector.tensor_copy(Bm16[:], Bm[:])
            bc_t = []
            for fc in range(NT):
                tp = ps_t.tile([128, 128], BF16, tag="tpb", bufs=1)
                nc.tensor.matmul(tp[:, 0:9], Bm16[:, fc * 128:(fc + 1) * 128],
                                 ident16[0:9, 0:9], is_transpose=True)
                bct = bcols.tile([128, 9], F32, tag="bc")
                nc.vector.tensor_copy(bct[:], tp[:, 0:9])
                bc_t.append(bct)
            vbrf = bcols.tile([1, E], F32, tag="vbrf", bufs=2)
            nc.sync.dma_start(vbrf[:], d_v_b.ap()[l: l + 1, :])
            vbr = bcols.tile([1, E], BF16, tag="vbr", bufs=2)
            nc.vector.tensor_copy(vbr[:], vbrf[:])
            return bc_t, vbr

        # prefetch layer 0 weights & bias columns; LN1 of layer 0
        wq_sb, wk_sb, wv_sb, wp_sb = load_qkvp(0)
        w1_sb = load_w1(0)
        w2_sb = load_w2(0)
        bc_t, vbr_l = load_bias_cols(0)
        vbias_row = vbr_l[:]
        ln_chunk(0, x_t, h_t)
        ln_chunk(1, x_t, h_t)

        for l in range(num_layers):
            # ---- QKV + vtok, both chunks (h_t already holds LN1 output) ----
            for c in range(2):
                c0, c1 = CHUNKS[c]
                for bidx, w_sb, dst in ((0, wq_sb, q_t), (1, wk_sb, k_t)):
                    for ot, ps in matmul_out(c, w_sb, h_t):
                        nc.vector.tensor_scalar_add(
                            dst[ot][:, c0:c1], ps[:], bc_t[ot][:, bidx:bidx + 1])
                # v computed directly token-major: out[tok, vdim] per token block
                # (stationary = h token-block, moving = Wv; bias via ones-row)
                for tb in range(4 * c, 4 * c + 4):
                    t0 = tb * 128
                    if tb % 2 == 0:
                        ps = ps_o.tile([128, 512], F32, tag="o")
                    else:
                        ps = ps_b.tile([128, 512], F32, tag="b")
                    for kc in range(NT):
                        nc.tensor.matmul(ps[:], h_t[kc][:, t0:t0 + 128],
                                         wv_sb[kc][:],
                                         start=(kc == 0), stop=False)
                    nc.tensor.matmul(ps[:], ones16_row[0:1, 0:128],
                                     vbias_row, start=False, stop=True)
                    nc.vector.tensor_copy(
                        vtok[tb].rearrange("p (h c) -> p h c", c=65)[:, :, 0:64],
                        ps[:].rearrange("p (h c) -> p h c", c=64))

            # prefetch next layer's MLP weights (DMAs flow during attention)
            if l + 1 < num_layers:
                nw1 = load_w1(l + 1)
                nw2 = load_w2(l + 1)

            # ---- attention, head-pipelined across both chunks ----
            def groups_for(c):
                if c == 0:
                    return [[(0, 0, 0, 512, True), (1, 512, 128, 384, True)],
                            [(2, 0, 256, 256, True), (3, 256, 384, 128, True)]]
                return [[(0, 0, 0, 512, False), (1, 512, 0, 512, False)],
                        [(2, 0, 0, 512, False), (3, 512, 0, 512, False)],
                        [(4, 0, 0, 512, True), (5, 512, 128, 384, True)],
                        [(6, 0, 256, 256, True), (7, 256, 384, 128, True)]]

            def emit_scores(c, hd):
                """scores+exp+mask for (c, hd); returns pt tiles w/ groups."""
                c0, c1 = CHUNKS[c]
                ht = hd // 2
                hp = (hd % 2) * 64
                q_h = q_t[ht][hp: hp + 64, :]
                k_h = k_t[ht][hp: hp + 64, :]
                pts = []
                for grp in groups_for(c):
                    wide = ps_sc.tile([128, 1024], F32, tag="sc")
                    gw = max(col + n for (_, col, _, n, _) in grp)
                    for (kt, col, qlo, n, diag) in grp:
                        nc.tensor.matmul(
                            wide[:, col:col + n],
                            k_h[:, kt * 128:(kt + 1) * 128],
                            q_h[:, c0 + qlo:c1],
                            start=True, stop=True)
                    pt = ptp.tile([128, 1024], BF16, tag="pt", bufs=8)
                    nc.scalar.activation(pt[:, 0:gw], wide[:, 0:gw],
                                         AF.Exp, scale=SCALE)
                    for (kt, col, qlo, n, diag) in grp:
                        if diag:
                            nc.gpsimd.affine_select(
                                out=pt[:, col:col + 128],
                                in_=pt[:, col:col + 128],
                                compare_op=ALU.is_ge, fill=0.0,
                                base=0, pattern=[[1, 128]],
                                channel_multiplier=-1)
                    pts.append((pt, grp))
                return pts

            def emit_attv(c, hd, pts):
                """att@V + normalize for (c, hd) from saved pt tiles."""
                c0, c1 = CHUNKS[c]
                ht = hd // 2
                hp = (hd % 2) * 64
                y_pst = ps_b.tile([128, 512], F32, tag="b")
                y_ps = y_pst[0:65, :]
                first = True
                for gi, (pt, grp) in enumerate(pts):
                    for (kt, col, qlo, n, diag) in grp:
                        nc.tensor.matmul(
                            y_ps[:, qlo:512],
                            vtok[kt][:, 65 * hd: 65 * hd + 65],
                            pt[:, col:col + n],
                            start=first, stop=(kt == grp[-1][0]
                                               and gi == len(pts) - 1))
                        first = False
                rec = recp.tile([1, 512], F32, tag="rr", bufs=2)
                nc.vector.reciprocal(rec[:], y_ps[64:65, :])
                rec_b = recp.tile([64, 512], F32, tag="rb", bufs=2)
                nc.gpsimd.partition_broadcast(rec_b[:], rec[:])
                nc.vector.tensor_mul(y_t[ht][hp: hp + 64, c0:c1],
                                     y_ps[0:64, :], rec_b[:])

            def emit_proj(c):
                c0, c1 = CHUNKS[c]
                for ot, ps in matmul_out(c, wp_sb, y_t):
                    nc.vector.scalar_tensor_tensor(
                        x_t[ot][:, c0:c1], ps[:], bc_t[ot][:, 3:4],
                        x_t[ot][:, c0:c1], ALU.add, ALU.add)

            pending = None
            for c in range(2):
                for hd in range(H):
                    pts = emit_scores(c, hd)
                    if pending is not None:
                        emit_attv(*pending)
                    pending = (c, hd, pts)
                    if c == 1 and hd == 2:
                        # y(c0) complete: fill PE idle during Act-bound exp
                        emit_proj(0)
            emit_attv(*pending)

            # prefetch next layer's QKV/proj weights + bias cols
            if l + 1 < num_layers:
                nwq, nwk, nwv, nwp = load_qkvp(l + 1)
                nbc, nvbr = load_bias_cols(l + 1)

            # ---- proj(c1) + LN2 + MLP; LN1(l+1) pipelined between chunks ----
            emit_proj(1)
            for c in range(2):
                ln_chunk(c, x_t, h_t)
                c0, c1 = CHUNKS[c]
                u_s = []
                for h16 in range(F // 128):
                    u_ps = ps_b.tile([128, 512], F32, tag="b")
                    for kc in range(NT):
                        nc.tensor.matmul(
                            u_ps[:], w1_sb[kc][:, h16 * 128:(h16 + 1) * 128],
                            h_t[kc][:, c0:c1],
                            start=(kc == 0), stop=(kc == NT - 1))
                    us = usp.tile([128, 512], BF16, tag="us")
                    b1col = bc_t[h16 % 4][:, 5 + h16 // 4: 6 + h16 // 4]
                    nc.scalar.activation(us[:], u_ps[:], AF.Gelu, bias=b1col)
                    u_s.append(us)
                for ot in range(NT):
                    if ot % 2 == 0:
                        ps = ps_o.tile([128, 512], F32, tag="o")
                    else:
                        ps = ps_b.tile([128, 512], F32, tag="b")
                    for h16 in range(F // 128):
                        nc.tensor.matmul(
                            ps[:], w2_sb[h16][:, ot * 128:(ot + 1) * 128],
                            u_s[h16][:], start=(h16 == 0), stop=(h16 == 15))
                    nc.vector.scalar_tensor_tensor(
                        x_t[ot][:, c0:c1], ps[:], bc_t[ot][:, 4:5],
                        x_t[ot][:, c0:c1], ALU.add, ALU.add)
                # LN of the NEXT stage for this chunk runs under the other
                # chunk's MLP matmuls (x[c] is final now).
                if l + 1 < num_layers or do_head:
                    ln_chunk(c, x_t, h_t)
            if l + 1 < num_layers:
                wq_sb, wk_sb, wv_sb, wp_sb = nwq, nwk, nwv, nwp
                w1_sb, w2_sb = nw1, nw2
                bc_t = nbc
                vbias_row = nvbr[:]

        # =================================================================
        # Prediction head (h_t holds LNf output already) + output transpose
        # =================================================================
        if do_head:
            pw_sb = []
            for kc in range(NT):
                wt = wqkv.tile([128, 72], BF16, tag="pw", bufs=4)
                nc.sync.dma_start(wt[:], d_pred_w.ap()[kc * 128:(kc + 1) * 128, :])
                pw_sb.append(wt)
            pb_rowf = const.tile([1, 72], F32)
            nc.sync.dma_start(pb_rowf[:], d_pred_b.ap().rearrange("(a e) -> a e", a=1))
            pb_row = const.tile([1, 72], BF16)
            nc.vector.tensor_copy(pb_row[:], pb_rowf[:])
            pb_col = const.tile([72, 1], F32)
            ptps = ps_t.tile([128, 128], BF16, tag="tpb", bufs=1)
            nc.tensor.matmul(ptps[0:72, 0:1], pb_row[:], ident16[0:1, 0:1],
                             is_transpose=True)
            nc.vector.tensor_copy(pb_col[:], ptps[0:72, 0:1])

            outT = const.tile([72, T], BF16)

            for c in range(2):
                # pred token range aligned to LN chunk: [2:512) / [512:1024)
                c0 = 2 if c == 0 else 512
                c1 = 512 if c == 0 else S
                n = c1 - c0
                ps = ps_o.tile([128, 512], F32, tag="o")
                for kc in range(NT):
                    nc.tensor.matmul(ps[0:72, 0:n], pw_sb[kc][:],
                                     h_t[kc][:, c0:c1], start=(kc == 0),
                                     stop=(kc == NT - 1))
                nc.scalar.activation(outT[:, c0 - 2: c1 - 2], ps[0:72, 0:n],
                                     AF.Identity, bias=pb_col[:, 0:1])

            for tt in range(NTT):
                ntt = min(128, T - tt * 128)
                tp = ps_t.tile([128, 128], BF16, tag="tpb", bufs=1)
                nc.tensor.matmul(tp[0:ntt, 0:72], outT[:, tt * 128: tt * 128 + ntt],
                                 ident16[0:72, 0:72], is_transpose=True)
                o_sb = scr.tile([128, 72], F32, tag="sa_tok")
                nc.vector.tensor_copy(o_sb[0:ntt, :], tp[0:ntt, 0:72])
                nc.sync.dma_start(d_out.ap()[tt * 128: tt * 128 + ntt, :],
                                  o_sb[0:ntt, :])

    nc.compile()
    return nc


_NC_CACHE = None


def _get_nc():
    global _NC_CACHE
    if _NC_CACHE is None:
        _NC_CACHE = build_nc()
    return _NC_CACHE


F32_PASSTHRU = [
    "sigma_w", "sigma_b", "tok_w", "tok_b", "goal_w", "goal_b", "pos_emb",
]


def make_in_maps(inputs):
    f32 = lambda k: np.asarray(inputs[k], np.float32)
    sa = f32("state_actions")
    goals = f32("goals")
    sigma = f32("sigma")
    shared = {n: np.ascontiguousarray(f32(n)) for n in F32_PASSTHRU}

    ln1_g, ln1_b = f32("ln1_g"), f32("ln1_b")
    ln2_g, ln2_b = f32("ln2_g"), f32("ln2_b")
    lnf_g, lnf_b = f32("lnf_g"), f32("lnf_b")

    def fold(w, b, g_l, b_l):
        # w [L,K,M], b [L,M]; LN gain/bias folded: w' = diag(g) w, b' = b + b_l @ w
        we = g_l[:, :, None] * w
        be = b + np.einsum("lk,lkm->lm", b_l, w)
        return we, be

    qw, qb = fold(f32("q_w"), f32("q_b"), ln1_g, ln1_b)
    kw, kb = fold(f32("k_w"), f32("k_b"), ln1_g, ln1_b)
    vw, vb = fold(f32("v_w"), f32("v_b"), ln1_g, ln1_b)
    w1, b1 = fold(f32("mlp_w1"), f32("mlp_b1"), ln2_g, ln2_b)
    predw = lnf_g[:, None] * f32("pred_w")
    predb = f32("pred_b") + lnf_b @ f32("pred_w")

    shared["q_be"], shared["k_be"], shared["v_be"] = qb, kb, vb
    shared["proj_be"], shared["mlp_b1e"], shared["mlp_b2e"] = \
        f32("proj_b"), b1, f32("mlp_b2")
    shared["pred_be"] = predb
    cvt = lambda a: np.ascontiguousarray(a.astype(BF))
    shared["q_w16"], shared["k_w16"], shared["v_w16"] = cvt(qw), cvt(kw), cvt(vw)
    shared["proj_w16"] = cvt(f32("proj_w"))
    shared["mlp_w116"], shared["mlp_w216"] = cvt(w1), cvt(f32("mlp_w2"))
    shared["pred_w16"] = cvt(predw)
    shared = {k: np.ascontiguousarray(v) for k, v in shared.items()}

    in_maps = []
    for b in range(B):
        m = dict(shared)
        m["state_actions"] = np.ascontiguousarray(sa[b])
        m["goals"] = np.ascontiguousarray(goals[b])
        m["sigma"] = np.ascontiguousarray(sigma[b: b + 1])
        in_maps.append(m)
    return in_maps


def run_spmd(inputs, **kwargs):
    nc = _get_nc()
    res = run_bass_kernel_spmd(nc, make_in_maps(inputs), list(range(B)), **kwargs)
    out = np.stack([res.results[c]["out"] for c in range(B)], axis=0)
    return out.astype(np.float32), res


def kernel(**inputs):
    out, _ = run_spmd(inputs)
    return out


# revision 16
# speedup vs baseline: 1.1304x; 1.1304x over previous
"""DiffusionGPT Trainium2 kernel (bf16, pipelined).

Data-parallel over batch: 8 batch elements -> 8 NeuronCores.  Activations
feature-major ([feature partitions, token free]) in bf16; weights host-
converted to bf16.  LayerNorm gains/biases are folded into the following
matmul weights on the host, so on-device LN is just (x - mean) * rstd.

PE-stream pipelining (the PE engine executes in program order, so emission
order is schedule):
  - attention is head-pipelined: att@V for head h-1 is emitted after
    scores+exp of head h, so the PE never head-of-line blocks on the exp.
  - proj(c0) is emitted in the middle of attention chunk 1 to fill the
    PE idle while the Act engine chews exp.
  - LN1 of layer l+1 runs between the two MLP chunks of layer l, hiding
    the LN stats chain under MLP matmuls.
  - v->token-major transposes use a 2-buffer PSUM ring so the PE does not
    serialize behind the DVE drain of each transpose.
PSUM drains carry the bias adds and run on DVE (att/gelu keep Act busy);
residual adds are fused (psum + bias) + x scalar_tensor_tensor ops.

Shapes: B=8, T=1022, S=1024, E=512, H=8 heads, D=64, F=2048, L=4 layers.
"""

import sys

sys.path.insert(0, "/opt/trn_rl_repo")

from contextlib import ExitStack

import numpy as np
import ml_dtypes

import concourse.bass as bass
import concourse.bacc as bacc
import concourse.tile as tile
from concourse import mybir
from concourse.bass_utils import run_bass_kernel_spmd
from concourse.masks import make_identity
from concourse import library_config

F32 = mybir.dt.float32
BF16 = mybir.dt.bfloat16
AF = mybir.ActivationFunctionType
ALU = mybir.AluOpType
BF = ml_dtypes.bfloat16

B = 8
T = 1022
S = 1024
E = 512
H = 8
D = 64
F = 2048
L = 4
NT = E // 128      # 4 feature tiles
NTT = S // 128     # 8 token tiles
LN_EPS = 1e-5
SCALE = 1.0 / 8.0  # 1/sqrt(D)

CHUNKS = ((0, 512), (512, 1024))


def build_nc(num_layers=L, do_head=True):
    nc = bacc.Bacc("TRN2", target_bir_lowering=False, debug=False)

    # ---- DRAM I/O ----
    d_sa = nc.dram_tensor("state_actions", [T, 72], F32, kind="ExternalInput")
    d_goals = nc.dram_tensor("goals", [1, 3], F32, kind="ExternalInput")
    d_sigma = nc.dram_tensor("sigma", [1], F32, kind="ExternalInput")
    d_sigma_w = nc.dram_tensor("sigma_w", [1, E], F32, kind="ExternalInput")
    d_sigma_b = nc.dram_tensor("sigma_b", [E], F32, kind="ExternalInput")
    d_tok_w = nc.dram_tensor("tok_w", [72, E], F32, kind="ExternalInput")
    d_tok_b = nc.dram_tensor("tok_b", [E], F32, kind="ExternalInput")
    d_goal_w = nc.dram_tensor("goal_w", [3, E], F32, kind="ExternalInput")
    d_goal_b = nc.dram_tensor("goal_b", [E], F32, kind="ExternalInput")
    d_pos = nc.dram_tensor("pos_emb", [1, S, E], F32, kind="ExternalInput")
    # LN-folded effective biases
    d_q_b = nc.dram_tensor("q_be", [L, E], F32, kind="ExternalInput")
    d_k_b = nc.dram_tensor("k_be", [L, E], F32, kind="ExternalInput")
    d_v_b = nc.dram_tensor("v_be", [L, E], F32, kind="ExternalInput")
    d_proj_b = nc.dram_tensor("proj_be", [L, E], F32, kind="ExternalInput")
    d_b1 = nc.dram_tensor("mlp_b1e", [L, F], F32, kind="ExternalInput")
    d_b2 = nc.dram_tensor("mlp_b2e", [L, E], F32, kind="ExternalInput")
    d_pred_b = nc.dram_tensor("pred_be", [72], F32, kind="ExternalInput")
    # host-converted, LN-gain-folded bf16 weights
    d_q_w = nc.dram_tensor("q_w16", [L, E, E], BF16, kind="ExternalInput")
    d_k_w = nc.dram_tensor("k_w16", [L, E, E], BF16, kind="ExternalInput")
    d_v_w = nc.dram_tensor("v_w16", [L, E, E], BF16, kind="ExternalInput")
    d_proj_w = nc.dram_tensor("proj_w16", [L, E, E], BF16, kind="ExternalInput")
    d_w1 = nc.dram_tensor("mlp_w116", [L, E, F], BF16, kind="ExternalInput")
    d_w2 = nc.dram_tensor("mlp_w216", [L, F, E], BF16, kind="ExternalInput")
    d_pred_w = nc.dram_tensor("pred_w16", [E, 72], BF16, kind="ExternalInput")
    d_out = nc.dram_tensor("out", [T, 72], F32, kind="ExternalOutput")

    with tile.TileContext(nc) as tc, ExitStack() as ctx:
        nc.gpsimd.load_library(library_config.attnmlp)

        const = ctx.enter_context(tc.tile_pool(name="const", bufs=1))
        big = ctx.enter_context(tc.tile_pool(name="big", bufs=1))
        wqkv = ctx.enter_context(tc.tile_pool(name="wqkv", bufs=28))
        w1p = ctx.enter_context(tc.tile_pool(name="w1p", bufs=6))
        w2p = ctx.enter_context(tc.tile_pool(name="w2p", bufs=18))
        bmat = ctx.enter_context(tc.tile_pool(name="bmat", bufs=2))
        bcols = ctx.enter_context(tc.tile_pool(name="bcols", bufs=8))
        ptp = ctx.enter_context(tc.tile_pool(name="ptp", bufs=8))
        usp = ctx.enter_context(tc.tile_pool(name="usp", bufs=16))
        rowp = ctx.enter_context(tc.tile_pool(name="rowp", bufs=5))
        bbp = ctx.enter_context(tc.tile_pool(name="bbp", bufs=2))
        recp = ctx.enter_context(tc.tile_pool(name="recp", bufs=2))
        scr = ctx.enter_context(tc.tile_pool(name="scr", bufs=4))

        # PSUM (16KB/partition): sc 2x4KB + b 2x2KB + o (2KB+.25) + t 2x.25
        ps_sc = ctx.enter_context(tc.tile_pool(name="ps_sc", bufs=2, space="PSUM"))
        ps_b = ctx.enter_context(tc.tile_pool(name="ps_b", bufs=2, space="PSUM"))
        ps_o = ctx.enter_context(tc.tile_pool(name="ps_o", bufs=1, space="PSUM"))
        ps_t = ps_o
        ps_st = ps_b

        # ---- constants ----
        ident = const.tile([128, 128], F32)
        make_identity(nc, ident[:])
        ident16 = const.tile([128, 128], BF16)
        nc.vector.tensor_copy(ident16[:], ident[:])
        ones64_16 = const.tile([128, 64], BF16)
        nc.gpsimd.memset(ones64_16[:], 1.0)
        ones_col16 = ones64_16[:, 0:1]

        # persistent activations (feature-major bf16)
        x_t = [big.tile([128, S], BF16, name=f"x{i}") for i in range(NT)]
        h_t = [big.tile([128, S], BF16, name=f"h{i}") for i in range(NT)]
        q_t = [big.tile([128, S], BF16, name=f"qa{i}") for i in range(NT)]
        k_t = [big.tile([128, S], BF16, name=f"ka{i}") for i in range(NT)]
        y_t = h_t
        sq_t = [big.tile([128, 512], BF16, name=f"sq{i}") for i in range(NT)]
        # v token-major with ones column per head: [128 tok, 8 heads x 65]
        vtok = [big.tile([128, H * 65], BF16, name=f"vtok{i}") for i in range(NTT)]
        for kt in range(NTT):
            vt3 = vtok[kt].rearrange("p (h c) -> p h c", c=65)
            nc.vector.tensor_copy(
                vt3[:, :, 64:65],
                ones64_16.rearrange("p (b c) -> p b c", c=1)[:, 0:H, :])

        # =================================================================
        # Embedding (f32 path, writes bf16 x)
        # =================================================================
        ones16_row = const.tile([1, 1024], BF16)
        nc.gpsimd.memset(ones16_row[:], 1.0)
        saT = const.tile([73, T], BF16)
        # engines can't start at partition 72; DMA can write any partition
        nc.sync.dma_start(saT[72:73, :], ones16_row[:, 0:T])
        for tt in range(NTT):
            ntt = min(128, T - tt * 128)
            sa_tok = scr.tile([128, 72], F32, tag="sa_tok")
            nc.sync.dma_start(sa_tok[0:ntt, :], d_sa[tt * 128: tt * 128 + ntt, :])
            sa16 = scr.tile([128, 72], BF16, tag="sa16", bufs=2)
            nc.vector.tensor_copy(sa16[0:ntt, :], sa_tok[0:ntt, :])
            tp = ps_t.tile([128, 128], BF16, tag="tpb", bufs=1)
            nc.tensor.matmul(tp[0:72, 0:ntt], sa16[0:ntt, 0:72],
                             ident16[0:ntt, 0:ntt], is_transpose=True)
            nc.vector.tensor_copy(saT[0:72, tt * 128: tt * 128 + ntt], tp[0:72, 0:ntt])

        tokw_f = const.tile([73, E], F32)
        nc.sync.dma_start(tokw_f[0:72, :], d_tok_w[:, :])
        nc.sync.dma_start(tokw_f[72:73, :], d_tok_b.ap().rearrange("(a e) -> a e", a=1))
        tokw_aug = const.tile([73, E], BF16)
        nc.vector.tensor_copy(tokw_aug[:], tokw_f[:])

        # sigma & goal columns: K=7 matmul per feature tile
        G_sf = const.tile([7, E], F32)
        nc.sync.dma_start(G_sf[0:1, :], d_sigma_w[:, :])
        nc.sync.dma_start(G_sf[1:2, :], d_sigma_b.ap().rearrange("(a e) -> a e", a=1))
        nc.sync.dma_start(G_sf[2:5, :], d_goal_w[:, :])
        nc.sync.dma_start(G_sf[5:6, :], d_goal_b.ap().rearrange("(a e) -> a e", a=1))
        nc.sync.dma_start(G_sf[6:7, :], d_pos.ap()[0, 0:1, :])
        G_sb = const.tile([7, E], BF16)
        nc.vector.tensor_copy(G_sb[:], G_sf[:])

        sig_sb = const.tile([1, 1], F32)
        nc.sync.dma_start(sig_sb[:], d_sigma.ap().rearrange("(a e) -> a e", a=1))
        lns = const.tile([1, 1], F32)
        nc.scalar.activation(lns[:], sig_sb[:], AF.Ln)
        sg_rowf = const.tile([1, 14], F32)
        nc.gpsimd.memset(sg_rowf[:], 0.0)
        nc.scalar.activation(sg_rowf[0:1, 0:1], lns[:], AF.Copy, scale=0.25)
        nc.gpsimd.memset(sg_rowf[0:1, 1:2], 1.0)
        g_row = const.tile([1, 3], F32)
        nc.sync.dma_start(g_row[:], d_goals[:, :])
        nc.vector.tensor_copy(sg_rowf[0:1, 9:12], g_row[:])
        nc.gpsimd.memset(sg_rowf[0:1, 12:14], 1.0)
        sg_row = const.tile([1, 14], BF16)
        nc.vector.tensor_copy(sg_row[:], sg_rowf[:])
        sg_rhs = const.tile([7, 2], BF16)
        for col in range(2):
            gtp = ps_t.tile([128, 128], BF16, tag="tpb", bufs=1)
            nc.tensor.matmul(gtp[0:7, 0:1], sg_row[0:1, col * 7:(col + 1) * 7],
                             ident16[0:1, 0:1], is_transpose=True)
            nc.vector.tensor_copy(sg_rhs[:, col: col + 1], gtp[0:7, 0:1])

        for fc in range(NT):
            sg_ps = ps_o.tile([128, 512], F32, tag="o")
            nc.tensor.matmul(sg_ps[0:128, 0:2], G_sb[:, fc * 128:(fc + 1) * 128],
                             sg_rhs[:], start=True, stop=True)
            nc.scalar.activation(x_t[fc][:, 0:2], sg_ps[0:128, 0:2], AF.Copy)

        # sa tokens -> x columns 2..1023
        for tt in range(NTT):
            ntt = min(128, T - tt * 128)
            xe_ps = ps_b.tile([128, 512], F32, tag="b")
            nc.tensor.matmul(xe_ps[0:ntt, :], saT[:, tt * 128: tt * 128 + ntt],
                             tokw_aug[:], start=True, stop=True)
            pos_sb = ptp.tile([128, E], F32, tag="pT", bufs=2)
            nc.sync.dma_start(pos_sb[0:ntt, :],
                              d_pos.ap()[0, tt * 128 + 1: tt * 128 + 1 + ntt, :])
            xe_tok = ptp.tile([128, E], BF16, tag="pT16", bufs=2)
            nc.vector.tensor_add(xe_tok[0:ntt, :], xe_ps[0:ntt, :], pos_sb[0:ntt, :])
            for fc in range(NT):
                tp = ps_t.tile([128, 128], BF16, tag="tpb", bufs=1)
                nc.tensor.matmul(tp[:, 0:ntt],
                                 xe_tok[0:ntt, fc * 128:(fc + 1) * 128],
                                 ident16[0:ntt, 0:ntt], is_transpose=True)
                nc.vector.tensor_copy(
                    x_t[fc][:, 2 + tt * 128: 2 + tt * 128 + ntt], tp[:, 0:ntt])

        # =================================================================
        # helpers
        # =================================================================
        def ln_chunk(c, src_t, dst_t):
            """Folded LayerNorm of chunk c: dst = (src - mean) * rstd."""
            c0, c1 = CHUNKS[c]
            for ti in range(NT):
                nc.vector.tensor_mul(sq_t[ti][:], src_t[ti][:, c0:c1],
                                     src_t[ti][:, c0:c1])
            s1t = ps_st.tile([128, 512], F32, tag="b")
            s1 = s1t[0:1, :]
            for ti in range(NT):
                nc.tensor.matmul(s1, ones_col16, src_t[ti][:, c0:c1],
                                 start=(ti == 0), stop=(ti == NT - 1))
            mean_row = rowp.tile([1, 512], F32, tag="rows")
            nc.vector.tensor_scalar_mul(mean_row[:], s1, 1.0 / E)
            s2t = ps_st.tile([128, 512], F32, tag="b")
            s2 = s2t[0:1, :]
            for ti in range(NT):
                nc.tensor.matmul(s2, ones_col16, sq_t[ti][:],
                                 start=(ti == 0), stop=(ti == NT - 1))
            m2_row = rowp.tile([1, 512], F32, tag="rows")
            nc.vector.tensor_scalar(m2_row[:], s2, 1.0 / E, LN_EPS,
                                    ALU.mult, ALU.add)
            msq = rowp.tile([1, 512], F32, tag="rows")
            nc.vector.tensor_mul(msq[:], mean_row[:], mean_row[:])
            nc.vector.tensor_sub(m2_row[:], m2_row[:], msq[:])
            w_row = rowp.tile([1, 512], F32, tag="rows")
            nc.vector.reciprocal(w_row[:], m2_row[:])
            rstd_row = rowp.tile([1, 512], F32, tag="rows")
            nc.scalar.activation(rstd_row[:], w_row[:], AF.Sqrt)
            mb = bbp.tile([128, 512], F32, tag="bb")
            nc.gpsimd.partition_broadcast(mb[:], mean_row[:])
            rb = bbp.tile([128, 512], F32, tag="bb")
            nc.gpsimd.partition_broadcast(rb[:], rstd_row[:])
            for ti in range(NT):
                t0 = sq_t[ti]  # reuse square scratch as LN scratch
                nc.vector.tensor_sub(t0[:], src_t[ti][:, c0:c1], mb[:])
                nc.vector.tensor_mul(dst_t[ti][:, c0:c1], t0[:], rb[:])

        def matmul_out(c, w_sb, in_t):
            """yield (ot, psum tile) for out = w^T @ in over chunk c.
            Alternates between the o/b PSUM rings for depth-3 pipelining."""
            c0, c1 = CHUNKS[c]
            for ot in range(NT):
                if ot % 2 == 0:
                    ps = ps_o.tile([128, 512], F32, tag="o")
                else:
                    ps = ps_b.tile([128, 512], F32, tag="b")
                for kc in range(NT):
                    nc.tensor.matmul(
                        ps[:], w_sb[kc][:, ot * 128:(ot + 1) * 128],
                        in_t[kc][:, c0:c1],
                        start=(kc == 0), stop=(kc == NT - 1))
                yield ot, ps

        # =================================================================
        # Transformer layers
        # =================================================================
        def load_qkvp(l):
            res = []
            for dw in (d_q_w, d_k_w, d_v_w, d_proj_w):
                tiles = []
                for kc in range(NT):
                    wt = wqkv.tile([128, E], BF16, tag="w")
                    nc.sync.dma_start(wt[:], dw.ap()[l, kc * 128:(kc + 1) * 128, :])
                    tiles.append(wt)
                res.append(tiles)
            return res

        def load_w1(l):
            tiles = []
            for kc in range(NT):
                wt = w1p.tile([128, F], BF16, tag="w1")
                nc.sync.dma_start(wt[:], d_w1.ap()[l, kc * 128:(kc + 1) * 128, :])
                tiles.append(wt)
            return tiles

        def load_w2(l):
            tiles = []
            for h16 in range(F // 128):
                wt = w2p.tile([128, E], BF16, tag="w2")
                nc.sync.dma_start(wt[:], d_w2.ap()[l, h16 * 128:(h16 + 1) * 128, :])
                tiles.append(wt)
            return tiles

        def load_bias_cols(l):
            # rows: 0 q_b, 1 k_b, 2 v_b, 3 proj_b, 4 mlp_b2, 5..8 mlp_b1
            Bm = bmat.tile([9, E], F32, tag="B")
            nc.sync.dma_start(Bm[0:1, :], d_q_b.ap()[l: l + 1, :])
            nc.sync.dma_start(Bm[1:2, :], d_k_b.ap()[l: l + 1, :])
            nc.sync.dma_start(Bm[2:3, :], d_v_b.ap()[l: l + 1, :])
            nc.sync.dma_start(Bm[3:4, :], d_proj_b.ap()[l: l + 1, :])
            nc.sync.dma_start(Bm[4:5, :], d_b2.ap()[l: l + 1, :])
            nc.sync.dma_start(Bm[5:9, :],
                              d_b1.ap()[l: l + 1, :].rearrange("a (b e) -> (a b) e", e=E))
            Bm16 = bmat.tile([9, E], BF16, tag="B16")
            nc.vector.tensor_copy(Bm16[:], Bm[:])
            bc_t = []
            for fc in range(NT):
                tp = ps_t.tile([128, 128], BF16, tag="tpb", bufs=1)
                nc.tensor.matmul(tp[:, 0:9], Bm16[:, fc * 128:(fc + 1) * 128],
                                 ident16[0:9, 0:9], is_transpose=True)
                bct = bcols.tile([128, 9], F32, tag="bc")
                nc.vector.tensor_copy(bct[:], tp[:, 0:9])
                bc_t.append(bct)
            vbrf = bcols.tile([1, E], F32, tag="vbrf", bufs=2)
            nc.sync.dma_start(vbrf[:], d_v_b.ap()[l: l + 1, :])
            vbr = bcols.tile([1, E], BF16, tag="vbr", bufs=2)
            nc.vector.tensor_copy(vbr[:], vbrf[:])
            return bc_t, vbr

        # prefetch layer 0 weights & bias columns; LN1 of layer 0
        wq_sb, wk_sb, wv_sb, wp_sb = load_qkvp(0)
        w1_sb = load_w1(0)
        w2_sb = load_w2(0)
        bc_t, vbr_l = load_bias_cols(0)
        vbias_row = vbr_l[:]
        ln_chunk(0, x_t, h_t)
        ln_chunk(1, x_t, h_t)

        def qkv_chunk(c, wq, wk, wv, bct, vbr):
            c0, c1 = CHUNKS[c]
            for bidx, w_sb, dst in ((0, wq, q_t), (1, wk, k_t)):
                for ot, ps in matmul_out(c, w_sb, h_t):
                    nc.vector.tensor_scalar_add(
                        dst[ot][:, c0:c1], ps[:], bct[ot][:, bidx:bidx + 1])
            # v computed directly token-major: out[tok, vdim] per token block
            # (stationary = h token-block, moving = Wv; bias via ones-row)
            for tb in range(4 * c, 4 * c + 4):
                t0 = tb * 128
                if tb % 2 == 0:
                    ps = ps_o.tile([128, 512], F32, tag="o")
                else:
                    ps = ps_b.tile([128, 512], F32, tag="b")
                for kc in range(NT):
                    nc.tensor.matmul(ps[:], h_t[kc][:, t0:t0 + 128],
                                     wv[kc][:], start=(kc == 0), stop=False)
                nc.tensor.matmul(ps[:], ones16_row[0:1, 0:128],
                                 vbr, start=False, stop=True)
                nc.vector.tensor_copy(
                    vtok[tb].rearrange("p (h c) -> p h c", c=65)[:, :, 0:64],
                    ps[:].rearrange("p (h c) -> p h c", c=64))

        qkv_chunk(0, wq_sb, wk_sb, wv_sb, bc_t, vbias_row)
        qkv_chunk(1, wq_sb, wk_sb, wv_sb, bc_t, vbias_row)

        for l in range(num_layers):
            # prefetch next layer's MLP weights (DMAs flow during attention)
            if l + 1 < num_layers:
                nw1 = load_w1(l + 1)
                nw2 = load_w2(l + 1)

            # ---- attention, head-pipelined across both chunks ----
            def groups_for(c):
                if c == 0:
                    return [[(0, 0, 0, 512, True), (1, 512, 128, 384, True)],
                            [(2, 0, 256, 256, True), (3, 256, 384, 128, True)]]
                return [[(0, 0, 0, 512, False), (1, 512, 0, 512, False)],
                        [(2, 0, 0, 512, False), (3, 512, 0, 512, False)],
                        [(4, 0, 0, 512, True), (5, 512, 128, 384, True)],
                        [(6, 0, 256, 256, True), (7, 256, 384, 128, True)]]

            def emit_scores(c, hd):
                """scores+exp+mask for (c, hd); returns pt tiles w/ groups."""
                c0, c1 = CHUNKS[c]
                ht = hd // 2
                hp = (hd % 2) * 64
                q_h = q_t[ht][hp: hp + 64, :]
                k_h = k_t[ht][hp: hp + 64, :]
                pts = []
                for grp in groups_for(c):
                    wide = ps_sc.tile([128, 1024], F32, tag="sc")
                    gw = max(col + n for (_, col, _, n, _) in grp)
                    for (kt, col, qlo, n, diag) in grp:
                        nc.tensor.matmul(
                            wide[:, col:col + n],
                            k_h[:, kt * 128:(kt + 1) * 128],
                            q_h[:, c0 + qlo:c1],
                            start=True, stop=True)
                    pt = ptp.tile([128, 1024], BF16, tag="pt", bufs=8)
                    nc.scalar.activation(pt[:, 0:gw], wide[:, 0:gw],
                                         AF.Exp, scale=SCALE)
                    for (kt, col, qlo, n, diag) in grp:
                        if diag:
                            nc.gpsimd.affine_select(
                                out=pt[:, col:col + 128],
                                in_=pt[:, col:col + 128],
                                compare_op=ALU.is_ge, fill=0.0,
                                base=0, pattern=[[1, 128]],
                                channel_multiplier=-1)
                    pts.append((pt, grp))
                return pts

            def emit_attv(c, hd, pts):
                """att@V + normalize for (c, hd) from saved pt tiles."""
                c0, c1 = CHUNKS[c]
                ht = hd // 2
                hp = (hd % 2) * 64
                y_pst = ps_b.tile([128, 512], F32, tag="b")
                y_ps = y_pst[0:65, :]
                first = True
                for gi, (pt, grp) in enumerate(pts):
                    for (kt, col, qlo, n, diag) in grp:
                        nc.tensor.matmul(
                            y_ps[:, qlo:512],
                            vtok[kt][:, 65 * hd: 65 * hd + 65],
                            pt[:, col:col + n],
                            start=first, stop=(kt == grp[-1][0]
                                               and gi == len(pts) - 1))
                        first = False
                rec = recp.tile([1, 512], F32, tag="rr", bufs=2)
                nc.vector.reciprocal(rec[:], y_ps[64:65, :])
                rec_b = recp.tile([64, 512], F32, tag="rb", bufs=2)
                nc.gpsimd.partition_broadcast(rec_b[:], rec[:])
                nc.vector.tensor_mul(y_t[ht][hp: hp + 64, c0:c1],
                                     y_ps[0:64, :], rec_b[:])

            def emit_proj(c):
                c0, c1 = CHUNKS[c]
                for ot, ps in matmul_out(c, wp_sb, y_t):
                    nc.vector.scalar_tensor_tensor(
                        x_t[ot][:, c0:c1], ps[:], bc_t[ot][:, 3:4],
                        x_t[ot][:, c0:c1], ALU.add, ALU.add)

            pending = None
            for c in range(2):
                for hd in range(H):
                    pts = emit_scores(c, hd)
                    if pending is not None:
                        emit_attv(*pending)
                    pending = (c, hd, pts)
                    if c == 1 and hd == 2:
                        # y(c0) complete: fill PE idle during Act-bound exp;
                        # LN2(c0) chain then runs under attention c1
                        emit_proj(0)
                        ln_chunk(0, x_t, h_t)
            emit_attv(*pending)

            # prefetch next layer's QKV/proj weights + bias cols
            if l + 1 < num_layers:
                nwq, nwk, nwv, nwp = load_qkvp(l + 1)
                nbc, nvbr = load_bias_cols(l + 1)

            # ---- proj(c1) + LN2(c1) + MLP; LN1(l+1) and QKV(l+1)
            # pipelined into the tail so PE never waits on an LN chain ----
            emit_proj(1)
            ln_chunk(1, x_t, h_t)
            for c in range(2):
                c0, c1 = CHUNKS[c]
                u_s = []
                for h16 in range(F // 128):
                    u_ps = ps_b.tile([128, 512], F32, tag="b")
                    for kc in range(NT):
                        nc.tensor.matmul(
                            u_ps[:], w1_sb[kc][:, h16 * 128:(h16 + 1) * 128],
                            h_t[kc][:, c0:c1],
                            start=(kc == 0), stop=(kc == NT - 1))
                    us = usp.tile([128, 512], BF16, tag="us")
                    b1col = bc_t[h16 % 4][:, 5 + h16 // 4: 6 + h16 // 4]
                    nc.scalar.activation(us[:], u_ps[:], AF.Gelu, bias=b1col)
                    u_s.append(us)
                for ot in range(NT):
                    if ot % 2 == 0:
                        ps = ps_o.tile([128, 512], F32, tag="o")
                    else:
                        ps = ps_b.tile([128, 512], F32, tag="b")
                    for h16 in range(F // 128):
                        nc.tensor.matmul(
                            ps[:], w2_sb[h16][:, ot * 128:(ot + 1) * 128],
                            u_s[h16][:], start=(h16 == 0), stop=(h16 == 15))
                    nc.vector.scalar_tensor_tensor(
                        x_t[ot][:, c0:c1], ps[:], bc_t[ot][:, 4:5],
                        x_t[ot][:, c0:c1], ALU.add, ALU.add)
                # LN of the NEXT stage for this chunk runs under the other
                # chunk's MLP / next QKV matmuls (x[c] is final now).
                if c == 0:
                    if l + 1 < num_layers or do_head:
                        ln_chunk(0, x_t, h_t)
                else:
                    if l + 1 < num_layers:
                        qkv_chunk(0, nwq, nwk, nwv, nbc, nvbr[:])
                    if l + 1 < num_layers or do_head:
                        ln_chunk(1, x_t, h_t)
                    if l + 1 < num_layers:
                        qkv_chunk(1, nwq, nwk, nwv, nbc, nvbr[:])
            if l + 1 < num_layers:
                wq_sb, wk_sb, wv_sb, wp_sb = nwq, nwk, nwv, nwp
                w1_sb, w2_sb = nw1, nw2
                bc_t = nbc
                vbias_row = nvbr[:]

        # =================================================================
        # Prediction head (h_t holds LNf output already) + output transpose
        # =================================================================
        if do_head:
            pw_sb = []
            for kc in range(NT):
                wt = wqkv.tile([128, 72], BF16, tag="pw", bufs=4)
                nc.sync.dma_start(wt[:], d_pred_w.ap()[kc * 128:(kc + 1) * 128, :])
                pw_sb.append(wt)
            pb_rowf = const.tile([1, 72], F32)
            nc.sync.dma_start(pb_rowf[:], d_pred_b.ap().rearrange("(a e) -> a e", a=1))
            pb_row = const.tile([1, 72], BF16)
            nc.vector.tensor_copy(pb_row[:], pb_rowf[:])
            pb_col = const.tile([72, 1], F32)
            ptps = ps_t.tile([128, 128], BF16, tag="tpb", bufs=1)
            nc.tensor.matmul(ptps[0:72, 0:1], pb_row[:], ident16[0:1, 0:1],
                             is_transpose=True)
            nc.vector.tensor_copy(pb_col[:], ptps[0:72, 0:1])

            outT = const.tile([72, T], BF16)

            for c in range(2):
                # pred token range aligned to LN chunk: [2:512) / [512:1024)
                c0 = 2 if c == 0 else 512
                c1 = 512 if c == 0 else S
                n = c1 - c0
                ps = ps_o.tile([128, 512], F32, tag="o")
                for kc in range(NT):
                    nc.tensor.matmul(ps[0:72, 0:n], pw_sb[kc][:],
                                     h_t[kc][:, c0:c1], start=(kc == 0),
                                     stop=(kc == NT - 1))
                nc.scalar.activation(outT[:, c0 - 2: c1 - 2], ps[0:72, 0:n],
                                     AF.Identity, bias=pb_col[:, 0:1])

            for tt in range(NTT):
                ntt = min(128, T - tt * 128)
                tp = ps_t.tile([128, 128], BF16, tag="tpb", bufs=1)
                nc.tensor.matmul(tp[0:ntt, 0:72], outT[:, tt * 128: tt * 128 + ntt],
                                 ident16[0:72, 0:72], is_transpose=True)
                o_sb = scr.tile([128, 72], F32, tag="sa_tok")
                nc.vector.tensor_copy(o_sb[0:ntt, :], tp[0:ntt, 0:72])
                nc.sync.dma_start(d_out.ap()[tt * 128: tt * 128 + ntt, :],
                                  o_sb[0:ntt, :])

    nc.compile()
    return nc


_NC_CACHE = None


def _get_nc():
    global _NC_CACHE
    if _NC_CACHE is None:
        _NC_CACHE = build_nc()
    return _NC_CACHE


F32_PASSTHRU = [
    "sigma_w", "sigma_b", "tok_w", "tok_b", "goal_w", "goal_b", "pos_emb",
]


def make_in_maps(inputs):
    f32 = lambda k: np.asarray(inputs[k], np.float32)
    sa = f32("state_actions")
    goals = f32("goals")
    sigma = f32("sigma")
    shared = {n: np.ascontiguousarray(f32(n)) for n in F32_PASSTHRU}

    ln1_g, ln1_b = f32("ln1_g"), f32("ln1_b")
    ln2_g, ln2_b = f32("ln2_g"), f32("ln2_b")
    lnf_g, lnf_b = f32("lnf_g"), f32("lnf_b")

    def fold(w, b, g_l, b_l):
        # w [L,K,M], b [L,M]; LN gain/bias folded: w' = diag(g) w, b' = b + b_l @ w
        we = g_l[:, :, None] * w
        be = b + np.einsum("lk,lkm->lm", b_l, w)
        return we, be

    qw, qb = fold(f32("q_w"), f32("q_b"), ln1_g, ln1_b)
    kw, kb = fold(f32("k_w"), f32("k_b"), ln1_g, ln1_b)
    vw, vb = fold(f32("v_w"), f32("v_b"), ln1_g, ln1_b)
    w1, b1 = fold(f32("mlp_w1"), f32("mlp_b1"), ln2_g, ln2_b)
    predw = lnf_g[:, None] * f32("pred_w")
    predb = f32("pred_b") + lnf_b @ f32("pred_w")

    shared["q_be"], shared["k_be"], shared["v_be"] = qb, kb, vb
    shared["proj_be"], shared["mlp_b1e"], shared["mlp_b2e"] = \
        f32("proj_b"), b1, f32("mlp_b2")
    shared["pred_be"] = predb
    cvt = lambda a: np.ascontiguousarray(a.astype(BF))
    shared["q_w16"], shared["k_w16"], shared["v_w16"] = cvt(qw), cvt(kw), cvt(vw)
    shared["proj_w16"] = cvt(f32("proj_w"))
    shared["mlp_w116"], shared["mlp_w216"] = cvt(w1), cvt(f32("mlp_w2"))
    shared["pred_w16"] = cvt(predw)
    shared = {k: np.ascontiguousarray(v) for k, v in shared.items()}

    in_maps = []
    for b in range(B):
        m = dict(shared)
        m["state_actions"] = np.ascontiguousarray(sa[b])
        m["goals"] = np.ascontiguousarray(goals[b])
        m["sigma"] = np.ascontiguousarray(sigma[b: b + 1])
        in_maps.append(m)
    return in_maps


def run_spmd(inputs, **kwargs):
    nc = _get_nc()
    res = run_bass_kernel_spmd(nc, make_in_maps(inputs), list(range(B)), **kwargs)
    out = np.stack([res.results[c]["out"] for c in range(B)], axis=0)
    return out.astype(np.float32), res


def kernel(**inputs):
    out, _ = run_spmd(inputs)
    return out
